# revision 1
# baseline (speedup 1.0000x reference)
"""Trainium2 Bass kernel for CustomPatchEmbedding (ragged patch gather + two projections).

Strategy (data-parallel over batch, 8 cores x 4 images):
  - Patch pixel rows are gathered straight from HBM images via SWDGE
    indirect DMA (one descriptor per contiguous patch row), landing as
    [patch, feature] tiles in SBUF with features in (c, dy, dx) order.
  - Gather indices are computed on-chip from the xy tensors (shift/add on
    DVE) plus small constant offset tables supplied as inputs.
  - TensorE transposes each 128-feature chunk to [feature, patch], then
    accumulates lhsT.T @ W^T chunks into PSUM ([patch, 256] fp32).
  - Bias is added from a partition-replicated bias tile; results DMA to DRAM.

kernel(**inputs) takes the FULL unsharded inputs and returns (32, 288, 256) f32.
"""
import sys
import numpy as np

sys.path.insert(0, "/opt/trn_rl_repo")

import concourse.bass as bass
import concourse.bacc as bacc
import concourse.mybir as mybir
import concourse.tile as tile
from concourse.masks import make_identity
from concourse.bass_utils import run_bass_kernel_spmd
from contextlib import ExitStack

# Problem constants (hardcoded per spec).
B, C, H, W = 32, 3, 512, 512
FP, CP = 16, 64
NF, NCO = 256, 32
D = 256
NCORES = 8
IPC = B // NCORES              # images per core
CHW = C * H * W                # 786432, per-image flat element count
NFLAT = IPC * CHW              # flat image elements per core
KF = C * FP * FP               # 768  fine features
KC = C * CP * CP               # 12288 coarse features
NROW_F = C * FP                # 48 gather rows per fine patch (c,dy)
NROW_C = C * CP                # 192 gather rows per coarse patch
P = 128

FDT = mybir.dt.float32
IDT = mybir.dt.int32

# Coarse gather is split into column-chunks of the index tile.
CJ = 24                        # idx columns per coarse gather chunk
NCHUNK_C = NROW_C // CJ        # 8 chunks
KPC = CJ * CP // P             # k-chunks (of 128) per coarse gather chunk = 12
NKF = KF // P                  # 6 fine k-chunks
NKC = KC // P                  # 96 coarse k-chunks


import os
VARIANT = os.environ.get("KVARIANT", "full")  # full | nogather | gatheronly


def _emit(nc, tc, t):
    """Emit the per-core Tile program. `t` maps tensor name -> dram handle."""
    no_gather = VARIANT == "nogather"
    gather_only = VARIANT == "gatheronly"
    with ExitStack() as ctx:
        const = ctx.enter_context(tc.tile_pool(name="const", bufs=1))
        small = ctx.enter_context(tc.tile_pool(name="small", bufs=1))
        gf_pool = ctx.enter_context(tc.tile_pool(name="gf", bufs=3))
        gc_pool = ctx.enter_context(tc.tile_pool(name="gc", bufs=3))
        wf_pool = ctx.enter_context(tc.tile_pool(name="wf", bufs=6))
        wc_pool = ctx.enter_context(tc.tile_pool(name="wc", bufs=14))
        lt_pool = ctx.enter_context(tc.tile_pool(name="lt", bufs=14))
        ob_pool = ctx.enter_context(tc.tile_pool(name="ob", bufs=3))
        ps_tp = ctx.enter_context(tc.tile_pool(name="ps_tp", bufs=4, space="PSUM"))
        ps_f = ctx.enter_context(tc.tile_pool(name="ps_f", bufs=2, space="PSUM"))
        ps_c = ctx.enter_context(tc.tile_pool(name="ps_c", bufs=1, space="PSUM"))

        # --- constants ---
        identity = const.tile([P, P], FDT)
        make_identity(nc, identity[:])
        tbl_f = const.tile([P, NROW_F], IDT)
        nc.sync.dma_start(tbl_f[:], t["tbl_f"][:])
        tbl_c = const.tile([P, NROW_C], IDT)
        nc.sync.dma_start(tbl_c[:], t["tbl_c"][:])
        bias_f = const.tile([P, D], FDT)
        nc.sync.dma_start(bias_f[:], t["bias_f"][:])
        bias_c = const.tile([P, D], FDT)
        nc.sync.dma_start(bias_c[:], t["bias_c"][:])

        # --- gather indices ---
        # coarse: one [128, 192] tile; partition p = (img, patch), col j = (c, dy)
        cxy = small.tile([P, 2], IDT)
        nc.sync.dma_start(cxy[:], t["coarse_xy"].ap().rearrange("b n two -> (b n) two"))
        cbase = small.tile([P, 1], IDT)
        nc.vector.tensor_scalar(
            out=cbase[:], in0=cxy[:, 1:2], scalar1=9, scalar2=None,
            op0=mybir.AluOpType.logical_shift_left,
        )
        nc.vector.tensor_tensor(
            out=cbase[:], in0=cbase[:], in1=cxy[:, 0:1], op=mybir.AluOpType.add
        )
        cidx = small.tile([P, NROW_C], IDT)
        nc.vector.tensor_tensor(
            out=cidx[:], in0=tbl_c[:], in1=cbase[:].to_broadcast([P, NROW_C]),
            op=mybir.AluOpType.add,
        )

        # fine: per (img b, half h) a [128, 48] tile
        fidx = []
        for b in range(IPC):
            for h in range(2):
                fxy = small.tile([P, 2], IDT, tag="fxy")
                nc.sync.dma_start(fxy[:], t["fine_xy"][b, h * P:(h + 1) * P, :])
                fb = small.tile([P, 1], IDT, tag="fb")
                nc.vector.tensor_scalar(
                    out=fb[:], in0=fxy[:, 1:2], scalar1=9, scalar2=None,
                    op0=mybir.AluOpType.logical_shift_left,
                )
                nc.vector.tensor_tensor(
                    out=fb[:], in0=fb[:], in1=fxy[:, 0:1], op=mybir.AluOpType.add
                )
                nc.vector.tensor_scalar(
                    out=fb[:], in0=fb[:], scalar1=b * CHW, scalar2=None,
                    op0=mybir.AluOpType.add,
                )
                fi = small.tile([P, NROW_F], IDT, tag=f"fidx{b}{h}")
                nc.vector.tensor_tensor(
                    out=fi[:], in0=tbl_f[:], in1=fb[:].to_broadcast([P, NROW_F]),
                    op=mybir.AluOpType.add,
                )
                fidx.append(fi)

        images = t["images"]
        out = t["out"]

        # --- fine branch: 8 groups of 128 patches ---
        # HW indirect DMA consumes ONE offset per destination partition, so each
        # (c,dy) row-column is its own gather instruction writing a 16-elem slice.
        for g in range(IPC * 2):
            b, h = divmod(g, 2)
            gt = gf_pool.tile([P, KF], FDT)
            if no_gather:
                nc.sync.dma_start(
                    gt[:],
                    images.ap().rearrange("(a f) one -> a (f one)", f=KF)[g * P:(g + 1) * P, :],
                )
            else:
                for j in range(NROW_F):
                    nc.gpsimd.indirect_dma_start(
                        out=gt[:, j * FP:(j + 1) * FP], out_offset=None, in_=images[:],
                        in_offset=bass.IndirectOffsetOnAxis(ap=fidx[g][:, j:j + 1], axis=0),
                    )
            if gather_only:
                nc.sync.dma_start(
                    out[b * (NF + NCO) + h * P:b * (NF + NCO) + (h + 1) * P, :],
                    gt[:, :D],
                )
                continue
            psum = ps_f.tile([P, D], FDT)
            lts = []
            for k in range(NKF):
                tp = ps_tp.tile([P, P], FDT, tag="tp")
                nc.tensor.transpose(
                    out=tp[:], in_=gt[:, k * P:(k + 1) * P], identity=identity[:]
                )
                lt = lt_pool.tile([P, P], FDT, tag="lt")
                nc.vector.tensor_copy(lt[:], tp[:])
                lts.append(lt)
            for k in range(NKF):
                wk = wf_pool.tile([P, D], FDT, tag="wf")
                nc.sync.dma_start(wk[:], t["wfT"][k * P:(k + 1) * P, :])
                nc.tensor.matmul(
                    out=psum[:], lhsT=lts[k][:], rhs=wk[:],
                    start=(k == 0), stop=(k == NKF - 1),
                )
            ob = ob_pool.tile([P, D], FDT, tag="ob")
            nc.vector.tensor_tensor(
                out=ob[:], in0=psum[:], in1=bias_f[:], op=mybir.AluOpType.add
            )
            nc.sync.dma_start(out[b * (NF + NCO) + h * P:b * (NF + NCO) + (h + 1) * P, :], ob[:])

        # --- coarse branch: one group of 128 patches, 96 k-chunks ---
        psum_c = None if gather_only else ps_c.tile([P, D], FDT)
        for cc in range(NCHUNK_C):
            gt = gc_pool.tile([P, CJ * CP], FDT)
            if no_gather:
                nc.sync.dma_start(
                    gt[:],
                    images.ap().rearrange("(a f) one -> a (f one)", f=CJ * CP)[cc * P:(cc + 1) * P, :],
                )
            else:
                for j in range(CJ):
                    nc.gpsimd.indirect_dma_start(
                        out=gt[:, j * CP:(j + 1) * CP], out_offset=None, in_=images[:],
                        in_offset=bass.IndirectOffsetOnAxis(
                            ap=cidx[:, cc * CJ + j:cc * CJ + j + 1], axis=0
                        ),
                    )
            if gather_only:
                nc.sync.dma_start(out[cc * P:(cc + 1) * P, :], gt[:, :D])
                continue
            lts = []
            for kk in range(KPC):
                tp = ps_tp.tile([P, P], FDT, tag="tp")
                nc.tensor.transpose(
                    out=tp[:], in_=gt[:, kk * P:(kk + 1) * P], identity=identity[:]
                )
                lt = lt_pool.tile([P, P], FDT, tag="lt")
                nc.vector.tensor_copy(lt[:], tp[:])
                lts.append(lt)
            for kk in range(KPC):
                k = cc * KPC + kk
                wk = wc_pool.tile([P, D], FDT, tag="wc")
                nc.sync.dma_start(wk[:], t["wcT"][k * P:(k + 1) * P, :])
                nc.tensor.matmul(
                    out=psum_c[:], lhsT=lts[kk][:], rhs=wk[:],
                    start=(k == 0), stop=(k == NKC - 1),
                )
        if not gather_only:
            oc = ob_pool.tile([P, D], FDT, tag="oc")
            nc.vector.tensor_tensor(
                out=oc[:], in0=psum_c[:], in1=bias_c[:], op=mybir.AluOpType.add
            )
            for b in range(IPC):
                nc.sync.dma_start(
                    out[b * (NF + NCO) + NF:b * (NF + NCO) + NF + NCO, :],
                    oc[b * NCO:(b + 1) * NCO, :],
                )


def build(reps: int = 1):
    nc = bacc.Bacc("TRN2", target_bir_lowering=False, debug=False)
    t = {
        "images": nc.dram_tensor("images", [NFLAT, 1], FDT, kind="ExternalInput"),
        "fine_xy": nc.dram_tensor("fine_xy", [IPC, NF, 2], IDT, kind="ExternalInput"),
        "coarse_xy": nc.dram_tensor("coarse_xy", [IPC, NCO, 2], IDT, kind="ExternalInput"),
        "wfT": nc.dram_tensor("wfT", [KF, D], FDT, kind="ExternalInput"),
        "wcT": nc.dram_tensor("wcT", [KC, D], FDT, kind="ExternalInput"),
        "bias_f": nc.dram_tensor("bias_f", [P, D], FDT, kind="ExternalInput"),
        "bias_c": nc.dram_tensor("bias_c", [P, D], FDT, kind="ExternalInput"),
        "tbl_f": nc.dram_tensor("tbl_f", [P, NROW_F], IDT, kind="ExternalInput"),
        "tbl_c": nc.dram_tensor("tbl_c", [P, NROW_C], IDT, kind="ExternalInput"),
        "out": nc.dram_tensor("out", [IPC * (NF + NCO), D], FDT, kind="ExternalOutput"),
    }
    with tile.TileContext(nc) as tc:
        for _ in range(reps):
            _emit(nc, tc, t)
    nc.compile()
    return nc


def host_tables():
    jf = np.arange(NROW_F)
    tbl_f = ((jf // FP) * H * W + (jf % FP) * W).astype(np.int32)
    tbl_f = np.repeat(tbl_f[None, :], P, axis=0)
    pc = np.arange(P)[:, None]
    jc = np.arange(NROW_C)[None, :]
    tbl_c = ((pc // NCO) * CHW + (jc // CP) * H * W + (jc % CP) * W).astype(np.int32)
    tbl_c = np.ascontiguousarray(tbl_c)
    return tbl_f, tbl_c


def make_in_maps(images, W_fine, b_fine, W_coarse, b_coarse, fine_xy, coarse_xy):
    images = np.asarray(images, dtype=np.float32)
    fine_xy = np.asarray(fine_xy, dtype=np.int32)
    coarse_xy = np.asarray(coarse_xy, dtype=np.int32)
    wfT = np.ascontiguousarray(np.asarray(W_fine, dtype=np.float32).T)
    wcT = np.ascontiguousarray(np.asarray(W_coarse, dtype=np.float32).T)
    bias_f = np.ascontiguousarray(np.repeat(np.asarray(b_fine, np.float32)[None, :], P, axis=0))
    bias_c = np.ascontiguousarray(np.repeat(np.asarray(b_coarse, np.float32)[None, :], P, axis=0))
    tbl_f, tbl_c = host_tables()
    in_maps = []
    for c in range(NCORES):
        sl = slice(c * IPC, (c + 1) * IPC)
        in_maps.append({
            "images": np.ascontiguousarray(images[sl]).reshape(NFLAT, 1),
            "fine_xy": np.ascontiguousarray(fine_xy[sl]),
            "coarse_xy": np.ascontiguousarray(coarse_xy[sl]),
            "wfT": wfT, "wcT": wcT,
            "bias_f": bias_f, "bias_c": bias_c,
            "tbl_f": tbl_f, "tbl_c": tbl_c,
        })
    return in_maps


_NC_CACHE = []


def _get_nc():
    if not _NC_CACHE:
        _NC_CACHE.append(build())
    return _NC_CACHE[0]


def run(inputs: dict, trace: bool = False):
    nc = _get_nc()
    in_maps = make_in_maps(**inputs)
    res = run_bass_kernel_spmd(nc, in_maps, list(range(NCORES)), trace=trace)
    outs = [
        np.asarray(res.results[c]["out"]).reshape(IPC, NF + NCO, D)
        for c in range(NCORES)
    ]
    return np.concatenate(outs, axis=0), res


def kernel(**inputs) -> np.ndarray:
    out, _ = run(inputs, trace=False)
    return out



# revision 5
# speedup vs baseline: 9.9207x; 9.9207x over previous
"""Trainium2 Bass kernel for CustomPatchEmbedding (ragged patch gather + two projections).

Strategy (data-parallel over batch, 8 cores x 4 images):
  - The host re-lays-out each image into sliding 16-row "slab" windows:
      Timg[b, c, y, x, dy] = img[b, c, y+dy, x]        (y in [0,512), dy in [0,16))
    In this layout a fine patch's channel slice (16x16 px) is ONE contiguous
    512B run, and a coarse patch's 16-row band (64x16 px) is ONE contiguous
    2KB run. SWDGE indirect DMA supports exactly one offset/descriptor per
    dest partition, so per core the whole gather is 36 instructions
    (8 fine groups x 3 channels + 12 coarse (c, band) blocks) of 128
    descriptors each, instead of 576 instructions of 64B rows.
  - The (c,dy,dx)->(c,dx,dy) feature reorder this induces is static and is
    folded into host-permuted, host-preswizzled bf16 weights.
  - Gather offsets are computed on the host from the xy tensors (int32).
  - Images and weights are bf16; PSUM accumulates fp32; output is fp32.
  - TensorE transposes each gathered 128-feature chunk; PSUM->SBUF cast
    copies alternate Vector/Scalar engines; matmuls accumulate in PSUM.

kernel(**inputs) takes the FULL unsharded inputs and returns (32, 288, 256) f32.
"""
import sys
import numpy as np

sys.path.insert(0, "/opt/trn_rl_repo")

import ml_dtypes
import concourse.bass as bass
import concourse.bacc as bacc
import concourse.mybir as mybir
import concourse.tile as tile
from concourse.masks import make_identity
from concourse.bass_utils import run_bass_kernel_spmd
from contextlib import ExitStack

# Problem constants (hardcoded per spec).
B, C, H, W = 32, 3, 512, 512
FP, CP = 16, 64
NF, NCO = 256, 32
D = 256
NCORES = 8
IPC = B // NCORES              # images per core
KF = C * FP * FP               # 768  fine features
KC = C * CP * CP               # 12288 coarse features
P = 128
NGRP_F = IPC * 2               # 8 fine groups of 128 patches
NKF = KF // P                  # 6 fine k-chunks
NKC = KC // P                  # 96 coarse k-chunks
NBLK_C = C * (CP // FP)        # 12 coarse gather blocks (c, 16-row band)
BLKC = FP * CP                 # 1024 elements per coarse gather block
SLAB = W * FP                  # 8192 elements per slab row
NSLAB = IPC * C * H * SLAB     # slab tensor elements per core (~50.3M)

FDT = mybir.dt.float32
BDT = mybir.dt.bfloat16
IDT = mybir.dt.int32
BF16 = ml_dtypes.bfloat16


def _emit(nc, tc, t):
    """Emit the per-core Tile program. `t` maps tensor name -> dram handle."""
    with ExitStack() as ctx:
        const = ctx.enter_context(tc.tile_pool(name="const", bufs=1))
        gf_pool = ctx.enter_context(tc.tile_pool(name="gf", bufs=6))
        gc_pool = ctx.enter_context(tc.tile_pool(name="gc", bufs=6))
        lt_pool = ctx.enter_context(tc.tile_pool(name="lt", bufs=4))
        ob_pool = ctx.enter_context(tc.tile_pool(name="ob", bufs=3))
        ps_tp = ctx.enter_context(tc.tile_pool(name="ps_tp", bufs=4, space="PSUM"))
        ps_f = ctx.enter_context(tc.tile_pool(name="ps_f", bufs=2, space="PSUM"))
        ps_c = ctx.enter_context(tc.tile_pool(name="ps_c", bufs=1, space="PSUM"))

        # --- constants ---
        identity = const.tile([P, P], BDT)
        make_identity(nc, identity[:])
        fidx = const.tile([P, NGRP_F * C], IDT)
        nc.sync.dma_start(fidx[:], t["fidx"][:])
        cidx = const.tile([P, NBLK_C], IDT)
        nc.sync.dma_start(cidx[:], t["cidx"][:])
        bias_f = const.tile([P, D], FDT)
        nc.sync.dma_start(bias_f[:], t["bias_f"][:])
        bias_c = const.tile([P, D], FDT)
        nc.sync.dma_start(bias_c[:], t["bias_c"][:])
        wf = const.tile([P, NKF * D], BDT)
        nc.sync.dma_start(wf[:], t["wf_sb"][:])
        wc = const.tile([P, NKC * D], BDT)
        nc.sync.dma_start(wc[:], t["wc_sb"][:])

        slabs = t["slabs"]
        out = t["out"]

        # --- fine branch: 8 groups of 128 patches, 3 gathers each ---
        for g in range(NGRP_F):
            b, h = divmod(g, 2)
            ft = []
            for _c in range(C):
                ftc = gf_pool.tile([P, FP * FP], BDT, tag="ft")
                ft.append(ftc)
            for c in range(C):
                nc.gpsimd.indirect_dma_start(
                    out=ft[c][:], out_offset=None, in_=slabs[:],
                    in_offset=bass.IndirectOffsetOnAxis(
                        ap=fidx[:, g * C + c:g * C + c + 1], axis=0
                    ),
                )
            psum = ps_f.tile([P, D], FDT)
            for k in range(NKF):
                c, half = divmod(k, 2)
                tp = ps_tp.tile([P, P], BDT, tag="tp")
                nc.tensor.transpose(
                    out=tp[:], in_=ft[c][:, half * P:(half + 1) * P],
                    identity=identity[:],
                )
                lt = lt_pool.tile([P, P], BDT, tag="lt")
                if k % 2 == 0:
                    nc.vector.tensor_copy(lt[:], tp[:])
                else:
                    nc.scalar.copy(lt[:], tp[:])
                nc.tensor.matmul(
                    out=psum[:], lhsT=lt[:], rhs=wf[:, k * D:(k + 1) * D],
                    start=(k == 0), stop=(k == NKF - 1),
                )
            ob = ob_pool.tile([P, D], FDT, tag="ob")
            nc.vector.tensor_tensor(
                out=ob[:], in0=psum[:], in1=bias_f[:], op=mybir.AluOpType.add
            )
            nc.sync.dma_start(
                out[b * (NF + NCO) + h * P:b * (NF + NCO) + (h + 1) * P, :], ob[:]
            )

        # --- coarse branch: one group of 128 patches, 12 gather blocks ---
        psum_c = ps_c.tile([P, D], FDT)
        for blk in range(NBLK_C):
            ct = gc_pool.tile([P, BLKC], BDT, tag="ct")
            nc.gpsimd.indirect_dma_start(
                out=ct[:], out_offset=None, in_=slabs[:],
                in_offset=bass.IndirectOffsetOnAxis(
                    ap=cidx[:, blk:blk + 1], axis=0
                ),
            )
            for kk in range(BLKC // P):
                k = blk * (BLKC // P) + kk
                tp = ps_tp.tile([P, P], BDT, tag="tp")
                nc.tensor.transpose(
                    out=tp[:], in_=ct[:, kk * P:(kk + 1) * P], identity=identity[:]
                )
                lt = lt_pool.tile([P, P], BDT, tag="lt")
                if k % 2 == 0:
                    nc.vector.tensor_copy(lt[:], tp[:])
                else:
                    nc.scalar.copy(lt[:], tp[:])
                nc.tensor.matmul(
                    out=psum_c[:], lhsT=lt[:], rhs=wc[:, k * D:(k + 1) * D],
                    start=(k == 0), stop=(k == NKC - 1),
                )
        oc = ob_pool.tile([P, D], FDT, tag="oc")
        nc.vector.tensor_tensor(
            out=oc[:], in0=psum_c[:], in1=bias_c[:], op=mybir.AluOpType.add
        )
        for b in range(IPC):
            nc.sync.dma_start(
                out[b * (NF + NCO) + NF:b * (NF + NCO) + NF + NCO, :],
                oc[b * NCO:(b + 1) * NCO, :],
            )


def build(reps: int = 1):
    nc = bacc.Bacc("TRN2", target_bir_lowering=False, debug=False)
    t = {
        "slabs": nc.dram_tensor("slabs", [NSLAB, 1], BDT, kind="ExternalInput"),
        "fidx": nc.dram_tensor("fidx", [P, NGRP_F * C], IDT, kind="ExternalInput"),
        "cidx": nc.dram_tensor("cidx", [P, NBLK_C], IDT, kind="ExternalInput"),
        "wf_sb": nc.dram_tensor("wf_sb", [P, NKF * D], BDT, kind="ExternalInput"),
        "wc_sb": nc.dram_tensor("wc_sb", [P, NKC * D], BDT, kind="ExternalInput"),
        "bias_f": nc.dram_tensor("bias_f", [P, D], FDT, kind="ExternalInput"),
        "bias_c": nc.dram_tensor("bias_c", [P, D], FDT, kind="ExternalInput"),
        "out": nc.dram_tensor("out", [IPC * (NF + NCO), D], FDT, kind="ExternalOutput"),
    }
    with tile.TileContext(nc) as tc:
        for _ in range(reps):
            _emit(nc, tc, t)
    nc.compile()
    return nc


def host_slabs(images_bf16):
    """images_bf16: (IPC, C, H, W) bf16 -> slab tensor (NSLAB,) bf16.

    Timg[b, c, y, x, dy] = img[b, c, y+dy, x]; y >= H-FP rows are zero-padded.
    """
    T = np.zeros((IPC, C, H, W, FP), dtype=BF16)
    sw = np.lib.stride_tricks.sliding_window_view(images_bf16, FP, axis=2)
    # sw[b, c, y, x, dy] = img[b, c, y+dy, x], y in [0, H-FP]
    T[:, :, :H - FP + 1] = sw
    return T.reshape(-1)


def host_indices(fine_xy, coarse_xy):
    """Per-core slab-gather offsets: fidx [128, 24], cidx [128, 12] (int32)."""
    # fine: col g*3+c; partition p = patch (g%2)*128+p of image g//2
    xy = fine_xy.reshape(NGRP_F, P, 2)                 # (8,128,2)
    b = np.arange(NGRP_F)[:, None] // 2                # (8,1)
    c = np.arange(C)[None, None, :]                    # (1,1,3)
    base = ((b[:, :, None] * C + c) * H + xy[:, :, 1:2]) * SLAB + xy[:, :, 0:1] * FP
    fidx = base.transpose(1, 0, 2).reshape(P, NGRP_F * C)
    # coarse: col c*4+j; partition p = (img p//32, patch p%32)
    cxy = coarse_xy.reshape(P, 2)
    bb = np.arange(P) // NCO                           # (128,)
    cc = np.arange(C)[None, :, None]                   # (1,3,1)
    jj = np.arange(CP // FP)[None, None, :]            # (1,1,4)
    cbase = ((bb[:, None, None] * C + cc) * H + cxy[:, 1:2, None] + jj * FP) * SLAB \
        + cxy[:, 0:1, None] * FP
    cidx = cbase.reshape(P, NBLK_C)
    return (np.ascontiguousarray(fidx.astype(np.int32)),
            np.ascontiguousarray(cidx.astype(np.int32)))


def host_weights(W_fine, W_coarse):
    """Permute features to slab order and swizzle to SBUF layout, bf16."""
    # fine: k = c*256 + dy*16 + dx  ->  k' = c*256 + dx*16 + dy
    wfT = np.asarray(W_fine, np.float32).T.reshape(C, FP, FP, D)      # [c,dy,dx,d]
    wfT = wfT.transpose(0, 2, 1, 3).reshape(KF, D)                    # [c,dx,dy,d]
    # coarse: k = c*4096 + dy64*64 + dx64 -> k' = ((c*4+j)*64 + dx64)*16 + dy
    wcT = np.asarray(W_coarse, np.float32).T.reshape(C, CP // FP, FP, CP, D)
    wcT = wcT.transpose(0, 1, 3, 2, 4).reshape(KC, D)                 # [c,j,dx64,dy,d]
    wf_sb = np.ascontiguousarray(
        wfT.reshape(NKF, P, D).transpose(1, 0, 2).reshape(P, NKF * D).astype(BF16))
    wc_sb = np.ascontiguousarray(
        wcT.reshape(NKC, P, D).transpose(1, 0, 2).reshape(P, NKC * D).astype(BF16))
    return wf_sb, wc_sb


def make_in_maps(images, W_fine, b_fine, W_coarse, b_coarse, fine_xy, coarse_xy):
    images = np.asarray(images, dtype=np.float32).astype(BF16)
    fine_xy = np.asarray(fine_xy, dtype=np.int64)
    coarse_xy = np.asarray(coarse_xy, dtype=np.int64)
    wf_sb, wc_sb = host_weights(W_fine, W_coarse)
    bias_f = np.ascontiguousarray(np.repeat(np.asarray(b_fine, np.float32)[None, :], P, axis=0))
    bias_c = np.ascontiguousarray(np.repeat(np.asarray(b_coarse, np.float32)[None, :], P, axis=0))
    in_maps = []
    for cid in range(NCORES):
        sl = slice(cid * IPC, (cid + 1) * IPC)
        fidx, cidx = host_indices(fine_xy[sl], coarse_xy[sl])
        in_maps.append({
            "slabs": host_slabs(images[sl]).reshape(NSLAB, 1),
            "fidx": fidx, "cidx": cidx,
            "wf_sb": wf_sb, "wc_sb": wc_sb,
            "bias_f": bias_f, "bias_c": bias_c,
        })
    return in_maps


_NC_CACHE = []


def _get_nc():
    if not _NC_CACHE:
        _NC_CACHE.append(build())
    return _NC_CACHE[0]


def run(inputs: dict, trace: bool = False):
    nc = _get_nc()
    in_maps = make_in_maps(**inputs)
    res = run_bass_kernel_spmd(nc, in_maps, list(range(NCORES)), trace=trace)
    outs = [
        np.asarray(res.results[c]["out"]).reshape(IPC, NF + NCO, D)
        for c in range(NCORES)
    ]
    return np.concatenate(outs, axis=0), res


def kernel(**inputs) -> np.ndarray:
    out, _ = run(inputs, trace=False)
    return out


# revision 7
# speedup vs baseline: 11.6076x; 1.1700x over previous
"""Trainium2 Bass kernel for CustomPatchEmbedding (ragged patch gather + two projections).

Strategy (data-parallel over batch, 8 cores x 4 images):
  - The host re-lays-out each image into channel-interleaved sliding 16-row
    slab windows:
      T2[b, y, x, c, dy] = img[b, c, y+dy, x]       (y in [0,512), dy in [0,16))
    In this layout one FULL fine patch (16x16x3c) is ONE contiguous 1536B run
    and one coarse 16-row band (64x16x3c) is ONE contiguous 6KB run. SWDGE
    indirect DMA supports exactly one offset/descriptor per dest partition, so
    the whole per-core gather is 12 instructions (8 fine groups + 4 coarse
    bands) of 128 large descriptors each.
  - The feature reorder this induces is static and folded into host-permuted,
    host-preswizzled bf16 weights.
  - Gather offsets are computed on the host from the xy tensors (int32).
  - Images and weights are bf16; PSUM accumulates fp32; output is fp32.
  - TensorE transposes each gathered 128-feature chunk; PSUM->SBUF cast copies
    alternate Vector/Scalar engines; matmuls accumulate in PSUM. The
    transpose/copy/matmul chain is software-pipelined (transposes run LAG
    chunks ahead) so the PE never stalls at its FIFO head waiting for a copy.

kernel(**inputs) takes the FULL unsharded inputs and returns (32, 288, 256) f32.
"""
import sys
import numpy as np

sys.path.insert(0, "/opt/trn_rl_repo")

import ml_dtypes
import concourse.bass as bass
import concourse.bacc as bacc
import concourse.mybir as mybir
import concourse.tile as tile
from concourse.masks import make_identity
from concourse.bass_utils import run_bass_kernel_spmd
from contextlib import ExitStack

# Problem constants (hardcoded per spec).
B, C, H, W = 32, 3, 512, 512
FP, CP = 16, 64
NF, NCO = 256, 32
D = 256
NCORES = 8
IPC = B // NCORES              # images per core
KF = C * FP * FP               # 768  fine features
KC = C * CP * CP               # 12288 coarse features
P = 128
NGRP_F = IPC * 2               # 8 fine groups of 128 patches
NKF = KF // P                  # 6 fine k-chunks
NKC = KC // P                  # 96 coarse k-chunks
NBND_C = CP // FP              # 4 coarse bands
BNDC = CP * C * FP             # 3072 elements per coarse band
KPB = BNDC // P                # 24 k-chunks per coarse band
XPITCH = C * FP                # 48 elements per x column in slab layout
SLAB = W * XPITCH              # 24576 elements per slab row
NSLAB = IPC * H * SLAB         # slab tensor elements per core (~50.3M)
LAG = 3                        # transpose->matmul software pipeline depth

FDT = mybir.dt.float32
BDT = mybir.dt.bfloat16
IDT = mybir.dt.int32
BF16 = ml_dtypes.bfloat16


def _emit(nc, tc, t):
    """Emit the per-core Tile program. `t` maps tensor name -> dram handle."""
    with ExitStack() as ctx:
        const = ctx.enter_context(tc.tile_pool(name="const", bufs=1))
        gf_pool = ctx.enter_context(tc.tile_pool(name="gf", bufs=3))
        gc_pool = ctx.enter_context(tc.tile_pool(name="gc", bufs=3))
        lt_pool = ctx.enter_context(tc.tile_pool(name="lt", bufs=2 * LAG + 2))
        ob_pool = ctx.enter_context(tc.tile_pool(name="ob", bufs=3))
        ps_tp = ctx.enter_context(tc.tile_pool(name="ps_tp", bufs=LAG + 2, space="PSUM"))
        ps_f = ctx.enter_context(tc.tile_pool(name="ps_f", bufs=2, space="PSUM"))
        ps_c = ctx.enter_context(tc.tile_pool(name="ps_c", bufs=1, space="PSUM"))

        # --- constants ---
        identity = const.tile([P, P], BDT)
        make_identity(nc, identity[:])
        fidx = const.tile([P, NGRP_F], IDT)
        nc.sync.dma_start(fidx[:], t["fidx"][:])
        cidx = const.tile([P, NBND_C], IDT)
        nc.sync.dma_start(cidx[:], t["cidx"][:])
        bias_f = const.tile([P, D], FDT)
        nc.sync.dma_start(bias_f[:], t["bias_f"][:])
        bias_c = const.tile([P, D], FDT)
        nc.sync.dma_start(bias_c[:], t["bias_c"][:])
        wf = const.tile([P, NKF * D], BDT)
        nc.sync.dma_start(wf[:], t["wf_sb"][:])
        wc = const.tile([P, NKC * D], BDT)
        nc.sync.dma_start(wc[:], t["wc_sb"][:])

        slabs = t["slabs"]
        out = t["out"]

        # Software pipeline: transpose+copy run LAG work-items ahead of the
        # matmul that consumes them, so the PE FIFO never stalls on a copy.
        pend = []
        ncopy = [0]

        def epilogue(psum, bias, rows):
            ob = ob_pool.tile([P, D], FDT, tag="ob")
            nc.vector.tensor_tensor(
                out=ob[:], in0=psum[:], in1=bias[:], op=mybir.AluOpType.add
            )
            for r0, r1, p0 in rows:
                nc.sync.dma_start(out[r0:r1, :], ob[p0:p0 + (r1 - r0), :])

        def push(src, kk, w, k, nk, psum, on_stop=None):
            tp = ps_tp.tile([P, P], BDT, tag="tp")
            nc.tensor.transpose(
                out=tp[:], in_=src[:, kk * P:(kk + 1) * P], identity=identity[:]
            )
            lt = lt_pool.tile([P, P], BDT, tag="lt")
            if ncopy[0] % 2 == 0:
                nc.vector.tensor_copy(lt[:], tp[:])
            else:
                nc.scalar.copy(lt[:], tp[:])
            ncopy[0] += 1
            pend.append((lt, w, k, nk, psum, on_stop))
            if len(pend) > LAG:
                fire(1)

        def fire(n):
            for _ in range(n):
                lt, w, k, nk, psum, on_stop = pend.pop(0)
                nc.tensor.matmul(
                    out=psum[:], lhsT=lt[:], rhs=w[:, k * D:(k + 1) * D],
                    start=(k == 0), stop=(k == nk - 1),
                )
                if k == nk - 1 and on_stop is not None:
                    on_stop()

        # --- fine branch: 8 groups of 128 patches, one gather each ---
        for g in range(NGRP_F):
            b, h = divmod(g, 2)
            gt = gf_pool.tile([P, KF], BDT, tag="gt")
            nc.gpsimd.indirect_dma_start(
                out=gt[:], out_offset=None, in_=slabs[:],
                in_offset=bass.IndirectOffsetOnAxis(
                    ap=fidx[:, g:g + 1], axis=0
                ),
            )
            psum = ps_f.tile([P, D], FDT, tag="psf")
            r0 = b * (NF + NCO) + h * P
            ep = (lambda ps, rows: lambda: epilogue(ps, bias_f, rows))(
                psum, [(r0, r0 + P, 0)])
            for k in range(NKF):
                push(gt, k, wf, k, NKF, psum, on_stop=ep if k == NKF - 1 else None)

        # --- coarse branch: one group of 128 patches, 4 band gathers ---
        psum_c = ps_c.tile([P, D], FDT)
        crows = [(b * (NF + NCO) + NF, b * (NF + NCO) + NF + NCO, b * NCO)
                 for b in range(IPC)]
        for bnd in range(NBND_C):
            ct = gc_pool.tile([P, BNDC], BDT, tag="ct")
            nc.gpsimd.indirect_dma_start(
                out=ct[:], out_offset=None, in_=slabs[:],
                in_offset=bass.IndirectOffsetOnAxis(
                    ap=cidx[:, bnd:bnd + 1], axis=0
                ),
            )
            for kk in range(KPB):
                k = bnd * KPB + kk
                push(ct, kk, wc, k, NKC, psum_c,
                     on_stop=(lambda: epilogue(psum_c, bias_c, crows))
                     if k == NKC - 1 else None)
        fire(len(pend))


def build(reps: int = 1):
    nc = bacc.Bacc("TRN2", target_bir_lowering=False, debug=False)
    t = {
        "slabs": nc.dram_tensor("slabs", [NSLAB, 1], BDT, kind="ExternalInput"),
        "fidx": nc.dram_tensor("fidx", [P, NGRP_F], IDT, kind="ExternalInput"),
        "cidx": nc.dram_tensor("cidx", [P, NBND_C], IDT, kind="ExternalInput"),
        "wf_sb": nc.dram_tensor("wf_sb", [P, NKF * D], BDT, kind="ExternalInput"),
        "wc_sb": nc.dram_tensor("wc_sb", [P, NKC * D], BDT, kind="ExternalInput"),
        "bias_f": nc.dram_tensor("bias_f", [P, D], FDT, kind="ExternalInput"),
        "bias_c": nc.dram_tensor("bias_c", [P, D], FDT, kind="ExternalInput"),
        "out": nc.dram_tensor("out", [IPC * (NF + NCO), D], FDT, kind="ExternalOutput"),
    }
    with tile.TileContext(nc) as tc:
        for _ in range(reps):
            _emit(nc, tc, t)
    nc.compile()
    return nc


def host_slabs(images_bf16):
    """images_bf16: (IPC, C, H, W) bf16 -> slab tensor (NSLAB,) bf16.

    T2[b, y, x, c, dy] = img[b, c, y+dy, x]; y >= H-FP rows are zero-padded.
    """
    T = np.zeros((IPC, H, W, C, FP), dtype=BF16)
    sw = np.lib.stride_tricks.sliding_window_view(images_bf16, FP, axis=2)
    # sw[b, c, y, x, dy] = img[b, c, y+dy, x], y in [0, H-FP]
    T[:, :H - FP + 1] = sw.transpose(0, 2, 3, 1, 4)
    return T.reshape(-1)


def host_indices(fine_xy, coarse_xy):
    """Per-core slab-gather offsets: fidx [128, 8], cidx [128, 4] (int32)."""
    # fine: col g; partition p = patch (g%2)*128+p of image g//2
    xy = fine_xy.reshape(NGRP_F, P, 2)                 # (8,128,2)
    b = np.arange(NGRP_F)[:, None] // 2                # (8,1)
    fidx = (b * H + xy[:, :, 1]) * SLAB + xy[:, :, 0] * XPITCH
    fidx = fidx.transpose(1, 0)
    # coarse: col j (16-row band); partition p = (img p//32, patch p%32)
    cxy = coarse_xy.reshape(P, 2)
    bb = np.arange(P) // NCO
    jj = np.arange(NBND_C)[None, :]
    cidx = ((bb[:, None] * H + cxy[:, 1:2] + jj * FP) * SLAB
            + cxy[:, 0:1] * XPITCH)
    return (np.ascontiguousarray(fidx.astype(np.int32)),
            np.ascontiguousarray(cidx.astype(np.int32)))


def host_weights(W_fine, W_coarse):
    """Permute features to slab order and swizzle to SBUF layout, bf16."""
    # fine: k = c*256 + dy*16 + dx  ->  k' = (dx*3 + c)*16 + dy
    wfT = np.asarray(W_fine, np.float32).T.reshape(C, FP, FP, D)      # [c,dy,dx,d]
    wfT = wfT.transpose(2, 0, 1, 3).reshape(KF, D)                    # [dx,c,dy,d]
    # coarse: k = c*4096 + (16j+dy)*64 + dx64 -> k' = ((j*64+dx64)*3 + c)*16 + dy
    wcT = np.asarray(W_coarse, np.float32).T.reshape(C, NBND_C, FP, CP, D)
    wcT = wcT.transpose(1, 3, 0, 2, 4).reshape(KC, D)                 # [j,dx,c,dy,d]
    wf_sb = np.ascontiguousarray(
        wfT.reshape(NKF, P, D).transpose(1, 0, 2).reshape(P, NKF * D).astype(BF16))
    wc_sb = np.ascontiguousarray(
        wcT.reshape(NKC, P, D).transpose(1, 0, 2).reshape(P, NKC * D).astype(BF16))
    return wf_sb, wc_sb


def make_in_maps(images, W_fine, b_fine, W_coarse, b_coarse, fine_xy, coarse_xy):
    images = np.asarray(images, dtype=np.float32).astype(BF16)
    fine_xy = np.asarray(fine_xy, dtype=np.int64)
    coarse_xy = np.asarray(coarse_xy, dtype=np.int64)
    wf_sb, wc_sb = host_weights(W_fine, W_coarse)
    bias_f = np.ascontiguousarray(np.repeat(np.asarray(b_fine, np.float32)[None, :], P, axis=0))
    bias_c = np.ascontiguousarray(np.repeat(np.asarray(b_coarse, np.float32)[None, :], P, axis=0))
    in_maps = []
    for cid in range(NCORES):
        sl = slice(cid * IPC, (cid + 1) * IPC)
        fidx, cidx = host_indices(fine_xy[sl], coarse_xy[sl])
        in_maps.append({
            "slabs": host_slabs(images[sl]).reshape(NSLAB, 1),
            "fidx": fidx, "cidx": cidx,
            "wf_sb": wf_sb, "wc_sb": wc_sb,
            "bias_f": bias_f, "bias_c": bias_c,
        })
    return in_maps


_NC_CACHE = []


def _get_nc():
    if not _NC_CACHE:
        _NC_CACHE.append(build())
    return _NC_CACHE[0]


def run(inputs: dict, trace: bool = False):
    nc = _get_nc()
    in_maps = make_in_maps(**inputs)
    res = run_bass_kernel_spmd(nc, in_maps, list(range(NCORES)), trace=trace)
    outs = [
        np.asarray(res.results[c]["out"]).reshape(IPC, NF + NCO, D)
        for c in range(NCORES)
    ]
    return np.concatenate(outs, axis=0), res


def kernel(**inputs) -> np.ndarray:
    out, _ = run(inputs, trace=False)
    return out
